# revision 24
# baseline (speedup 1.0000x reference)
"""AttentionBlock3D kernel for 8 Trainium2 NeuronCores.

Problem (hardcoded): x [2, 256, 16, 16, 16] fp32, GroupNorm(8 groups) ->
qkv 1x1 conv -> 8-head attention over S=4096 -> proj -> residual.

Sharding: sequence-parallel. Core i handles batch b = i//4 and the
s-chunk [1024*(i%4), 1024*(i%4+1)) of the flattened spatial dim. Every
core recomputes GroupNorm stats and full k/v for its batch; q /
attention rows / proj / output are computed only for the core's own
s-chunk, so the host-side unshard is a pure concatenation.

V3 pipeline per core:
  - x streamed in 1K-column chunks on two DMA queues; GroupNorm stats
    via ACT Identity/Square with accum_out per chunk (overlaps the DMA).
  - qkv biases folded into the PSUM->SBUF casts on ACT (per-partition
    bias AP), v bias folded algebraically into the proj bias
    (proj_w @ bv added to proj_b), so there are no K=1 bias matmuls.
  - v and k(ct0) issued column-chunk-interleaved right after the
    affine; k(ct1) chunks are interleaved into attention heads 0-3.
  - scores transposed (scoresT[t, s] = k^T q) in fp16, K=32 row tiling
    (3 concurrent tile_position row-groups); ACT exp reads the 3-bank
    PSUM group and writes fp8e4 expS to SBUF directly, with a constant
    log2-shift of -4 in the exp bias (cancels in softmax, keeps values
    under ACT's fp8 saturation at 256).
  - attn @ v as fp8 DoubleRow matmuls (two 128-t blocks per
    instruction, K_eff=256, 0.5 cycles/row); the fp8 ones-column yields
    the denominator Z as row 32 of each per-half U tile.
  - k3/q3 replication for head h+1 prefetched during head h (k3 on the
    gpsimd queue, q3 on the sync queue).
  - Z normalization deferred and batched: Z rows DMA'd to zall[16,512],
    one reciprocal_approx_fast, select-mask matmuls broadcast 1/Z, two
    in-place DVE multiplies normalize attnout before proj.
"""

import numpy as np

B, C, S = 2, 256, 4096
NH, HD, G = 8, 32, 8
EPS = 1e-5
SC = 1024          # s-chunk length per core
NCORES = 8
NTB = S // 128     # 32 t-blocks
SCALE = HD ** -0.5
GN_N = (C // G) * S

LOG2E = float(np.log2(np.e))
P8 = float(8.0 * LOG2E * SCALE)       # q prescale: scores arrive as 8*log2(exp)
EXP_SCALE = float(np.log(2.0) / 8.0)  # ACT exp scale undoing the prescale
EXP_BIAS = float(-4.5 * np.log(2.0))  # constant 2^-4.5 shift, cancels in softmax

# DVE fp8-exp bit trick: with the half-quantum (4.5 log2) shift, the
# round-to-nearest-8 of ps gives a floor decomposition with a single
# smooth branch: bits = r8(ps) + C2 + fk*(A8*fk + B8), fk = ps - r8(ps).
A8 = 0.04283219
B8 = 0.99208890
C2V = 8.0 + 11.31312237
MVAL = 1.5 * 2 ** 26
DVE_GROUPS = (1, 4, 6, 9)   # group indices per (head, half) offloaded to DVE

_cache = {}

EXP8 = None


def _register_exp8():
    global EXP8
    if EXP8 is not None:
        return EXP8
    import re
    from concourse import dve_ops
    from concourse.dve_spec import Spec, Src0, C0, C1, C2, C3
    from concourse.dve_ops import DveOp, _spill_c3_to_src1

    def _ref(in0, in1, c0, c1, c2):
        ps = in0.astype(np.float32)
        M = np.float32(MVAL)
        wmk = ((ps + M).astype(np.float32) - M).astype(np.float32)
        fk = (ps - wmk).astype(np.float32)
        return (wmk + np.float32(c2)).astype(np.float32) + fk * (
            np.float32(c0) * fk + np.float32(c1))

    _u0 = Src0 + C3
    _wmk = _u0 - C3
    _fk = Src0 - _wmk
    _body = (_wmk + C2) + _fk * (C0 * _fk + C1)
    op = DveOp("EXP8_ANT",
               Spec(body=_spill_c3_to_src1(_body), reference=_ref),
               subdim=False, uops_sha={})
    dve_ops.OPS.append(op)
    dve_ops._SUB_OPCODE_FOR_NAME[op.name] = (
        dve_ops._CUSTOM_DVE_ROW_BASE + len(dve_ops.OPS) - 1)
    dve_ops.CUSTOM_DVE_SPECS[op.name] = op.spec
    for ver in ("v3", "v4"):
        try:
            op.compile(ver)
        except ValueError as e:
            m = re.search(r'uops_sha\["' + ver + r'"\]="([0-9a-f]+)"', str(e))
            op.uops_sha[ver] = m.group(1)
    EXP8 = op
    return op


def _build_nc():
    import concourse.bass as bass
    import concourse.bacc as bacc
    import concourse.tile as tile
    from concourse import mybir
    from concourse.masks import make_identity

    f32 = mybir.dt.float32
    f16 = mybir.dt.float16
    f8 = mybir.dt.float8e4
    u8 = mybir.dt.uint8
    AF = mybir.ActivationFunctionType
    ALU = mybir.AluOpType
    AX = mybir.AxisListType
    DR = mybir.MatmulPerfMode.DoubleRow
    _register_exp8()

    nc = bacc.Bacc()
    dx = nc.declare_dram_parameter("x_full", [C, S], f32, isOutput=False)
    dxq = nc.declare_dram_parameter("xq", [C, SC], f32, isOutput=False)
    dgw = nc.declare_dram_parameter("gn_w", [C], f32, isOutput=False)
    dgb = nc.declare_dram_parameter("gn_b", [C], f32, isOutput=False)
    dqkvw = nc.declare_dram_parameter("qkv_w", [3 * C, C], f32, isOutput=False)
    dqkvb = nc.declare_dram_parameter("qkv_b", [3 * C], f32, isOutput=False)
    dpw = nc.declare_dram_parameter("proj_w", [C, C], f32, isOutput=False)
    dpb = nc.declare_dram_parameter("proj_b", [C], f32, isOutput=False)
    dsel = nc.declare_dram_parameter("selm", [16, 512], f16, isOutput=False)
    dout = nc.declare_dram_parameter("out", [C, SC], f32, isOutput=True)

    from contextlib import ExitStack
    with tile.TileContext(nc) as tc, ExitStack() as ctx:
        singles = ctx.enter_context(tc.tile_pool(name="singles", bufs=1))
        # PSUM pools: 2x3 + 2 = 8 banks exactly.
        ps_pool = ctx.enter_context(tc.tile_pool(name="ps", bufs=2, space="PSUM"))
        u_pool = ctx.enter_context(tc.tile_pool(name="u", bufs=2, space="PSUM"))
        kq = ctx.enter_context(tc.tile_pool(name="kq", bufs=2))
        vt_pool = ctx.enter_context(tc.tile_pool(name="vtp", bufs=1))
        # preamble-scoped pools (released before expS allocates)
        # h_sb outlives the preamble: k(ct1) chunks stream under heads 0-3
        hp = ctx.enter_context(tc.tile_pool(name="hp", bufs=2))
        pre = ExitStack()
        xp = pre.enter_context(tc.tile_pool(name="xp", bufs=2))
        wstage = pre.enter_context(tc.tile_pool(name="wstage", bufs=2))

        ones = singles.tile([128, 128], f32, tag="ones", name="ones")
        nc.vector.memset(ones, 1.0)
        ebias = singles.tile([128, 1], f32, tag="ebias", name="ebias")
        nc.vector.memset(ebias, EXP_BIAS)
        ident = singles.tile([128, 128], f32, tag="ident", name="ident")
        make_identity(nc, ident)
        sel = singles.tile([16, 2, 2, 128], f16, tag="sel", name="sel")
        nc.sync.dma_start(out=sel, in_=dsel[:, :].rearrange(
            "k (ct half c) -> k ct half c", ct=2, half=2))

        # ---- x / xq DMA (sync queue, column-chunked) + weights (gpsimd) ----
        x_sb = [xp.tile([128, S], f32, tag="x", name="x") for _ in range(2)]
        xq_sb = [xp.tile([128, SC], f32, tag="xq", name="xq") for _ in range(2)]
        NCC = 4
        for cc in range(NCC):
            for ct in range(2):
                nc.sync.dma_start(
                    out=x_sb[ct][:, 1024 * cc:1024 * (cc + 1)],
                    in_=dx[128 * ct:128 * (ct + 1), 1024 * cc:1024 * (cc + 1)])
        for ct in range(2):
            nc.sync.dma_start(out=xq_sb[ct], in_=dxq[128 * ct:128 * (ct + 1), :])

        wle = [wstage.tile([128, C], f32, tag=f"wle{rt}", name=f"wle{rt}")
               for rt in range(8)]
        for rt in range(6):
            nc.gpsimd.dma_start(out=wle[rt], in_=dqkvw[128 * rt:128 * (rt + 1), :])
        for rt in range(2):
            nc.gpsimd.dma_start(out=wle[6 + rt], in_=dpw[128 * rt:128 * (rt + 1), :])

        # small parameter columns
        gw = [singles.tile([128, 1], f32, tag=f"gw{i}", name=f"gw{i}") for i in range(2)]
        gb = [singles.tile([128, 1], f32, tag=f"gb{i}", name=f"gb{i}") for i in range(2)]
        pb = [singles.tile([128, 1], f32, tag=f"pb{i}", name=f"pb{i}") for i in range(2)]
        bqc = [singles.tile([128, 1], f32, tag=f"bqc{i}", name=f"bqc{i}") for i in range(2)]
        bkc = [singles.tile([128, 1], f32, tag=f"bkc{i}", name=f"bkc{i}") for i in range(2)]
        bvc = [singles.tile([128, 1], f32, tag=f"bvc{i}", name=f"bvc{i}") for i in range(2)]
        for ct in range(2):
            nc.gpsimd.dma_start(out=gw[ct], in_=dgw[128 * ct:128 * (ct + 1)])
            nc.gpsimd.dma_start(out=gb[ct], in_=dgb[128 * ct:128 * (ct + 1)])
            nc.gpsimd.dma_start(out=pb[ct], in_=dpb[128 * ct:128 * (ct + 1)])
            nc.gpsimd.dma_start(out=bqc[ct], in_=dqkvb[128 * ct:128 * (ct + 1)])
            nc.gpsimd.dma_start(out=bkc[ct], in_=dqkvb[C + 128 * ct:C + 128 * (ct + 1)])
            nc.gpsimd.dma_start(out=bvc[ct], in_=dqkvb[2 * C + 128 * ct:2 * C + 128 * (ct + 1)])

        # ---- vT_aug zero-init (fp16-bitcast memset) ----
        vt = vt_pool.tile([128, NTB, NH, 64], f8, tag="vt", name="vt")
        nc.vector.memset(vt.bitcast(f16), 0.0)
        nc.vector.memset(vt[:, :, :, 32:33], 1.0)

        # ---- weight transposes (PE) ----
        wqkvT = [singles.tile([128, 3 * C], f16, tag=f"wqkvT{i}", name=f"wqkvT{i}") for i in range(2)]
        wpT = [singles.tile([128, C], f16, tag=f"wpT{i}", name=f"wpT{i}") for i in range(2)]
        for rt in range(6):
            for ct in range(2):
                pt = ps_pool.tile([128, 1536], f32, tag="ps", name="ps")
                nc.tensor.transpose(pt[:, 0:128], wle[rt][:, 128 * ct:128 * (ct + 1)], ident)
                nc.vector.tensor_copy(
                    out=wqkvT[ct][:, 128 * rt:128 * (rt + 1)], in_=pt[:, 0:128])
        for rt in range(2):
            for ct in range(2):
                pt = ps_pool.tile([128, 1536], f32, tag="ps", name="ps")
                nc.tensor.transpose(pt[:, 0:128], wle[6 + rt][:, 128 * ct:128 * (ct + 1)], ident)
                nc.vector.tensor_copy(
                    out=wpT[ct][:, 128 * rt:128 * (rt + 1)], in_=pt[:, 0:128])

        # ---- GroupNorm stats on ACT (accum_out), chunked ----
        h_sb = [hp.tile([128, S], f16, tag="h", name="h") for _ in range(2)]
        ssum = [wstage.tile([128, NCC], f32, tag=f"ssum{i}", name=f"ssum{i}") for i in range(2)]
        ssq = [wstage.tile([128, NCC], f32, tag=f"ssq{i}", name=f"ssq{i}") for i in range(2)]
        for cc in range(NCC):
            for ct in range(2):
                scratch = h_sb[ct][:, 1024 * cc:1024 * (cc + 1)]
                xcc = x_sb[ct][:, 1024 * cc:1024 * (cc + 1)]
                nc.scalar.activation(out=scratch, in_=xcc, func=AF.Identity,
                                     accum_out=ssum[ct][:, cc:cc + 1])
                nc.scalar.activation(out=scratch, in_=xcc, func=AF.Square,
                                     accum_out=ssq[ct][:, cc:cc + 1])
        stats = [wstage.tile([128, 2], f32, tag=f"st{i}", name=f"st{i}") for i in range(2)]
        for ct in range(2):
            nc.vector.tensor_reduce(out=stats[ct][:, 0:1], in_=ssum[ct],
                                    axis=AX.X, op=ALU.add)
            nc.vector.tensor_reduce(out=stats[ct][:, 1:2], in_=ssq[ct],
                                    axis=AX.X, op=ALU.add)
        st_ps = u_pool.tile([1, 512], f32, tag="u", name="st_ps")
        for ct in range(2):
            nc.tensor.transpose(st_ps[0:1, 128 * ct:128 * (ct + 1)],
                                stats[ct][:, 0:1], ident)
            nc.tensor.transpose(st_ps[0:1, 256 + 128 * ct:256 + 128 * (ct + 1)],
                                stats[ct][:, 1:2], ident)
        gstats = singles.tile([1, 16], f32, tag="gstats", name="gstats")
        nc.vector.tensor_reduce(
            out=gstats,
            in_=st_ps.rearrange("p (k g c) -> p k g c", k=2, g=G),
            axis=AX.X, op=ALU.add)
        mu = singles.tile([1, G], f32, tag="mu", name="mu")
        varv = singles.tile([1, G], f32, tag="varv", name="varv")
        rstd = singles.tile([1, G], f32, tag="rstd", name="rstd")
        mrs = singles.tile([1, G], f32, tag="mrs", name="mrs")
        eps_sb2 = singles.tile([128, 1], f32, tag="eps2", name="eps2")
        nc.vector.memset(eps_sb2, EPS)
        nc.vector.tensor_scalar_mul(out=mu, in0=gstats[:, 0:G], scalar1=1.0 / GN_N)
        nc.vector.tensor_scalar_mul(out=varv, in0=gstats[:, G:2 * G], scalar1=1.0 / GN_N)
        musq = singles.tile([1, G], f32, tag="musq", name="musq")
        nc.vector.tensor_mul(out=musq, in0=mu, in1=mu)
        nc.vector.tensor_sub(out=varv, in0=varv, in1=musq)
        vb_ps = u_pool.tile([128, 512], f32, tag="u", name="vb_ps")
        nc.tensor.matmul(vb_ps[:, 0:G], ones[0:1, 0:128], varv,
                         start=True, stop=True)
        sdb = singles.tile([128, G], f32, tag="sdb", name="sdb")
        nc.scalar.activation(out=sdb, in_=vb_ps[:, 0:G], func=AF.Sqrt,
                             bias=eps_sb2)
        nc.vector.reciprocal(out=rstd, in_=sdb[0:1, :])
        nc.vector.tensor_mul(out=mrs, in0=mu, in1=rstd)

        # broadcast rstd/mrs to per-channel A, Bb
        A = [singles.tile([128, 1], f32, tag=f"A{i}", name=f"A{i}") for i in range(2)]
        Bb = [singles.tile([128, 1], f32, tag=f"B{i}", name=f"B{i}") for i in range(2)]
        for ct in range(2):
            arep = u_pool.tile([128, 2], f32, tag="u", name="arep")
            for g4 in range(4):
                g = 4 * ct + g4
                nc.tensor.matmul(
                    arep[32 * g4:32 * (g4 + 1), 0:1], ones[0:1, 0:32],
                    rstd[:, g:g + 1], start=True, stop=True,
                    tile_position=(0, 32 * g4))
                nc.tensor.matmul(
                    arep[32 * g4:32 * (g4 + 1), 1:2], ones[0:1, 0:32],
                    mrs[:, g:g + 1], start=True, stop=True,
                    tile_position=(0, 32 * g4))
            nc.vector.tensor_mul(out=A[ct], in0=arep[:, 0:1], in1=gw[ct])
            tmp = wstage.tile([128, 1], f32, tag="tmpB", name="tmpB")
            nc.vector.tensor_mul(out=tmp, in0=arep[:, 1:2], in1=gw[ct])
            nc.vector.tensor_sub(out=Bb[ct], in0=gb[ct], in1=tmp)

        # scaled q bias (bq * P8) for the ACT q cast
        bqP = [singles.tile([128, 1], f32, tag=f"bqP{i}", name=f"bqP{i}") for i in range(2)]
        for ct in range(2):
            nc.vector.tensor_scalar(out=bqP[ct], in0=bqc[ct], scalar1=P8,
                                    scalar2=None, op0=ALU.mult)
        # proj bias' = proj_b + proj_w @ bv (v bias folded past attention)
        bvc16 = [singles.tile([128, 1], f16, tag=f"bvc16{i}", name=f"bvc16{i}") for i in range(2)]
        for kc in range(2):
            nc.vector.tensor_copy(out=bvc16[kc], in_=bvc[kc])
        pbp = [singles.tile([128, 1], f32, tag=f"pbp{i}", name=f"pbp{i}") for i in range(2)]
        for ct in range(2):
            ppb = u_pool.tile([128, 2], f32, tag="u", name="ppb")
            for kc in range(2):
                nc.tensor.matmul(ppb[:, 0:1],
                                 wpT[kc][:, 128 * ct:128 * (ct + 1)], bvc16[kc],
                                 start=(kc == 0), stop=(kc == 1))
            nc.vector.tensor_add(out=pbp[ct], in0=pb[ct], in1=ppb[:, 0:1])

        # ---- hq affine + xpb + q ----
        hq_sb = [hp.tile([128, SC], f16, tag="hq", name="hq") for _ in range(2)]
        xpb = [singles.tile([128, SC], f32, tag=f"xpb{i}", name=f"xpb{i}") for i in range(2)]
        for ct in range(2):
            nc.scalar.activation(out=hq_sb[ct], in_=xq_sb[ct], func=AF.Identity,
                                 bias=Bb[ct], scale=A[ct])
            nc.scalar.activation(out=xpb[ct], in_=xq_sb[ct], func=AF.Identity,
                                 bias=pbp[ct])
        q_sb = [kq.tile([128, SC], f16, tag="q", name="q") for _ in range(2)]
        k_sb = [kq.tile([128, S], f16, tag="k", name="k") for _ in range(2)]
        for ct in range(2):
            pq = ps_pool.tile([128, 1536], f32, tag="ps", name="ps")
            for n in range(2):
                for kc in range(2):
                    nc.tensor.matmul(
                        pq[:, 512 * n:512 * (n + 1)],
                        wqkvT[kc][:, 128 * ct:128 * (ct + 1)],
                        hq_sb[kc][:, 512 * n:512 * (n + 1)],
                        start=(kc == 0), stop=(kc == 1))
            nc.scalar.activation(out=q_sb[ct], in_=pq[:, 0:SC], func=AF.Identity,
                                 bias=bqP[ct], scale=P8)

        # ---- h affine + v + k(ct0), column-chunk interleaved ----
        def k_chunk(ct, chunk):
            pk = ps_pool.tile([128, 1536], f32, tag="ps", name="ps")
            for n in range(2):
                cl = 1024 * chunk + 512 * n
                for kc in range(2):
                    nc.tensor.matmul(
                        pk[:, 512 * n:512 * (n + 1)],
                        wqkvT[kc][:, C + 128 * ct:C + 128 * (ct + 1)],
                        h_sb[kc][:, cl:cl + 512],
                        start=(kc == 0), stop=(kc == 1))
            nc.scalar.activation(
                out=k_sb[ct][:, 1024 * chunk:1024 * (chunk + 1)],
                in_=pk[:, 0:1024], func=AF.Identity, bias=bkc[ct])

        for cc in range(NCC):
            for ct in range(2):
                nc.scalar.activation(
                    out=h_sb[ct][:, 1024 * cc:1024 * (cc + 1)],
                    in_=x_sb[ct][:, 1024 * cc:1024 * (cc + 1)],
                    func=AF.Identity, bias=Bb[ct], scale=A[ct])
            for tb in range(8 * cc, 8 * (cc + 1)):
                pv = u_pool.tile([128, 512], f32, tag="u", name="u")
                for kc in range(2):
                    nc.tensor.matmul(
                        pv[:, 0:256],
                        h_sb[kc][:, 128 * tb:128 * (tb + 1)],
                        wqkvT[kc][:, 2 * C:3 * C],
                        start=(kc == 0), stop=(kc == 1))
                nc.vector.tensor_copy(
                    out=vt[:, tb, :, 0:32],
                    in_=pv[:, 0:256].rearrange("p (h d) -> p h d", h=NH))
            k_chunk(0, cc)

        # ---- attention ----
        pre.close()
        exps_pool = ctx.enter_context(tc.tile_pool(name="exps", bufs=1))
        rep = ctx.enter_context(tc.tile_pool(name="rep", bufs=2))
        usb_pool = ctx.enter_context(tc.tile_pool(name="usb", bufs=2))
        expS = exps_pool.tile([128, 2, NTB, 528], f8, tag="expS", name="expS")
        m2t = singles.tile([128, 1], f32, tag="m2t", name="m2t")
        nc.vector.memset(m2t, MVAL)
        zall = singles.tile([16, 512], f16, tag="zall", name="zall")
        attnout = [kq.tile([128, SC], f16, tag="ao", name="ao") for _ in range(2)]
        groups = [list(range(i, min(i + 3, NTB))) for i in range(0, NTB, 3)]
        NPAIR = NTB // 2

        def issue_rep(h):
            kt, kr = h // 4, 32 * (h % 4)
            k3 = rep.tile([96, S], f16, tag="k3", name="k3")
            q3 = rep.tile([96, SC], f16, tag="q3", name="q3")
            for i in range(3):
                nc.gpsimd.dma_start(out=k3[32 * i:32 * (i + 1), :],
                                    in_=k_sb[kt][kr:kr + 32, :])
                nc.sync.dma_start(out=q3[32 * i:32 * (i + 1), :],
                                  in_=q_sb[kt][kr:kr + 32, :])
            return k3, q3

        rep_tiles = issue_rep(0)
        for h in range(NH):
            kt, kr = h // 4, 32 * (h % 4)
            k3, q3 = rep_tiles
            if h < 4:
                k_chunk(1, h)   # stream k(ct1) under heads 0-3
            for half in range(2):
                if half == 1 and h + 1 < NH:
                    rep_tiles = issue_rep(h + 1)
                U = u_pool.tile([64, 512], f32, tag="u", name="u")
                done = 0
                for gi, grp in enumerate(groups):
                    ng = len(grp)
                    ps = ps_pool.tile([128, 1536], f32, tag="ps", name="ps")
                    for i, tb in enumerate(grp):
                        nc.tensor.matmul(
                            ps[:, 512 * i:512 * (i + 1)],
                            k3[32 * i:32 * (i + 1), 128 * tb:128 * (tb + 1)],
                            q3[32 * i:32 * (i + 1), 512 * half:512 * (half + 1)],
                            start=True, stop=True, tile_position=(32 * i, 0))
                    if gi in DVE_GROUPS:
                        nc.vector._custom_dve(
                            EXP8,
                            out=expS[:, half, grp[0]:grp[0] + ng,
                                     0:512].bitcast(u8),
                            in0=ps[:, 0:512 * ng].rearrange(
                                "p (t s) -> p t s", s=512),
                            in1=m2t, s0=A8, s1=B8, imm2=C2V)
                    else:
                        nc.scalar.activation(
                            out=expS[:, half, grp[0]:grp[0] + ng, 0:512],
                            in_=ps[:, 0:512 * ng].rearrange("p (t s) -> p t s", s=512),
                            func=AF.Exp, scale=EXP_SCALE, bias=ebias)
                    # issue AV pairs one group late so exp latency is hidden
                    if gi >= 1:
                        ready = (groups[gi - 1][-1] + 1) // 2
                        for p in range(done, ready):
                            nc.tensor.matmul(
                                U,
                                vt[:, 2 * p:2 * p + 2, h, :],
                                expS[:, half, 2 * p:2 * p + 2, 0:512],
                                start=(p == 0), stop=(p == NPAIR - 1),
                                perf_mode=DR)
                        done = ready
                for p in range(done, NPAIR):
                    nc.tensor.matmul(
                        U,
                        vt[:, 2 * p:2 * p + 2, h, :],
                        expS[:, half, 2 * p:2 * p + 2, 0:512],
                        start=(p == 0), stop=(p == NPAIR - 1),
                        perf_mode=DR)
                # rows 0:32 = v-weighted sums, row 32 = Z (ones column)
                u_sb = usb_pool.tile([33, 512], f16, tag="usb", name="usb")
                nc.vector.tensor_copy(out=u_sb, in_=U[0:33, :])
                nc.gpsimd.dma_start(
                    out=attnout[kt][kr:kr + 32, 512 * half:512 * (half + 1)],
                    in_=u_sb[0:32, :])
                nc.gpsimd.dma_start(
                    out=zall[2 * h + half:2 * h + half + 1, :],
                    in_=u_sb[32:33, :])

        # ---- batched 1/Z + normalization ----
        zallf = singles.tile([16, 512], f32, tag="zallf", name="zallf")
        nc.vector.tensor_copy(out=zallf, in_=zall)
        zinv = singles.tile([16, 512], f32, tag="zinv", name="zinv")
        nc.vector.reciprocal_approx_fast(out=zinv, in_=zallf)
        zinv16 = singles.tile([16, 512], f16, tag="zinv16", name="zinv16")
        nc.vector.tensor_copy(out=zinv16, in_=zinv)

        out_sb = [usb_pool.tile([128, SC], f32, tag="osb", name="osb") for _ in range(2)]
        for ct in range(2):
            zrep = u_pool.tile([128, 512], f32, tag="u", name="u")
            zrep2 = u_pool.tile([128, 512], f32, tag="u", name="u")
            nc.tensor.matmul(zrep, sel[:, ct, 0, :], zinv16,
                             start=True, stop=True)
            nc.tensor.matmul(zrep2, sel[:, ct, 1, :], zinv16,
                             start=True, stop=True)
            nc.vector.tensor_mul(out=attnout[ct][:, 0:512],
                                 in0=attnout[ct][:, 0:512], in1=zrep)
            nc.vector.tensor_mul(out=attnout[ct][:, 512:1024],
                                 in0=attnout[ct][:, 512:1024], in1=zrep2)

        # ---- proj + residual ----
        for ct in range(2):
            pp = ps_pool.tile([128, 1536], f32, tag="ps", name="ps")
            for n in range(2):
                for kc in range(2):
                    nc.tensor.matmul(
                        pp[:, 512 * n:512 * (n + 1)],
                        wpT[kc][:, 128 * ct:128 * (ct + 1)],
                        attnout[kc][:, 512 * n:512 * (n + 1)],
                        start=(kc == 0), stop=(kc == 1))
            nc.vector.tensor_add(out=out_sb[ct], in0=pp[:, 0:SC], in1=xpb[ct])
            nc.sync.dma_start(out=dout[128 * ct:128 * (ct + 1), :],
                              in_=out_sb[ct])

    nc.finalize()
    return nc


def kernel(x, gn_w, gn_b, qkv_w, qkv_b, proj_w, proj_b):
    import sys
    if "/opt/trn_rl_repo" not in sys.path:
        sys.path.insert(0, "/opt/trn_rl_repo")
    from concourse.bass_utils import run_bass_kernel_spmd

    if "nc" not in _cache:
        _cache["nc"] = _build_nc()
    nc = _cache["nc"]

    x = np.ascontiguousarray(np.asarray(x, dtype=np.float32))
    xf = x.reshape(B, C, S)
    if "selm" not in _cache:
        selm = np.zeros((16, 2, 2, 128), np.float16)
        for ct in range(2):
            for half in range(2):
                for h4 in range(4):
                    selm[8 * ct + 2 * h4 + half, ct, half,
                         32 * h4:32 * (h4 + 1)] = 1.0
        _cache["selm"] = selm.reshape(16, 512)
    in_maps = []
    for i in range(NCORES):
        b, sc = i // 4, SC * (i % 4)
        in_maps.append({
            "x_full": xf[b],
            "xq": np.ascontiguousarray(xf[b][:, sc:sc + SC]),
            "gn_w": np.asarray(gn_w, np.float32),
            "gn_b": np.asarray(gn_b, np.float32),
            "qkv_w": np.asarray(qkv_w, np.float32),
            "qkv_b": np.asarray(qkv_b, np.float32),
            "proj_w": np.asarray(proj_w, np.float32),
            "proj_b": np.asarray(proj_b, np.float32),
            "selm": _cache["selm"],
        })
    res = run_bass_kernel_spmd(nc, in_maps, list(range(NCORES))).results
    out = np.empty((B, C, S), np.float32)
    for i in range(NCORES):
        b, sc = i // 4, SC * (i % 4)
        out[b][:, sc:sc + SC] = res[i]["out"]
    return out.reshape(B, C, 16, 16, 16)
